# revision 31
# baseline (speedup 1.0000x reference)
"""Multi-head attention (B=4, S=2048, D=1024, H=16, Dh=64) on 8 TRN2 NeuronCores.

Sharding: core c -> batch b = c//2, head-group g = c%2 (8 heads, output cols
g*512:(g+1)*512).  Host ships x pre-transposed ([D, S]) and weights in bf16;
each core runs attention for its (batch, 8 heads) slice; host concatenates the
per-core [2048, 512] outputs.

Per-core kernel (bf16 compute, f32 accumulation):
  - qTz = per-head zero-padded q (head rows in its pair slot, zeros in the
    other head's rows): scores run as K=128 matmuls (lhsT = packed kT pair,
    the foreign head's k rows hit q zeros).  K=64 same-row-group matmuls
    serialize on LDWEIGHTS; full-K rotating weights pipeline cleanly.
  - v natural = xT.T @ Wv (+bias via K=1 ones-matmul), augmented with a
    ones-column per head so the AV matmul also produces softmax denominators.
  - per head, per sq-tile: scoresT[sk,sq] K=128 single-shot matmuls into
    double-buffered [128,1024] f32 PSUM; exp on ScalarE (1024 wide, the
    1/sqrt(1024) scale folded into the activation; scores are O(1), no
    max-subtraction needed); out_hT/denoms accumulate in PSUM over sk chunks.
  - PE-transpose [65,128] slabs -> natural [128,64|denom] -> reciprocal +
    per-partition scalar multiply -> out rows.
  - v/qk projections and the previous head's transpose/normalize tail are
    interleaved into the attention chunk stream so the TensorE's slack under
    the ScalarE-bound exp is spent on useful work (and HAM stays warm).
"""

import numpy as np
import ml_dtypes
from contextlib import ExitStack

import concourse.bass as bass
import concourse.bacc as bacc
import concourse.mybir as mybir
import concourse.tile as tile
from concourse.bass_utils import run_bass_kernel_spmd
from concourse.masks import make_identity

F32 = mybir.dt.float32
BF16 = mybir.dt.bfloat16

B, S, D = 4, 2048, 1024
H, DH = 16, 64
N_CORES = 8
HPC = 8          # heads per core
DPC = HPC * DH   # output cols per core = 512
SCALE = 1.0 / 32.0  # 1/sqrt(D)

KD = D // 128    # 8 contraction chunks over d_in
NS = S // 128    # 16 sequence chunks
MB = DPC // 128  # 4 partition blocks (head pairs)
NT = S // 1024   # 2 sq tiles

_CACHE = {}


def _build_program():
    nc = bacc.Bacc("TRN2", target_bir_lowering=False, debug=False)

    xt_ext = nc.dram_tensor("xt", [D, S], BF16, kind="ExternalInput").ap()
    wq_ext = nc.dram_tensor("wq", [D, DPC], BF16, kind="ExternalInput").ap()
    wk_ext = nc.dram_tensor("wk", [D, DPC], BF16, kind="ExternalInput").ap()
    wv_ext = nc.dram_tensor("wv", [D, DPC], BF16, kind="ExternalInput").ap()
    bq_ext = nc.dram_tensor("bq", [DPC], F32, kind="ExternalInput").ap()
    bk_ext = nc.dram_tensor("bk", [DPC], F32, kind="ExternalInput").ap()
    bv_ext = nc.dram_tensor("bv", [DPC], F32, kind="ExternalInput").ap()
    out_ext = nc.dram_tensor("out", [S, DPC], F32, kind="ExternalOutput").ap()

    with tile.TileContext(nc) as tc, ExitStack() as ctx:
        singles = ctx.enter_context(tc.tile_pool(name="singles", bufs=1))

        # --- DMAs: tiny bias vectors first (they gate the first projection
        # copybacks and would otherwise queue behind 7MB of weights), then x,
        # then weights in use order ---
        bq_col = []
        bk_col = []
        for m in range(MB):
            t = singles.tile([128, 1], F32, tag=f"bq{m}", name=f"bq{m}")
            nc.sync.dma_start(
                out=t, in_=bq_ext[m * 128:(m + 1) * 128].rearrange("(p o) -> p o", o=1)
            )
            bq_col.append(t)
            t = singles.tile([128, 1], F32, tag=f"bk{m}", name=f"bk{m}")
            nc.sync.dma_start(
                out=t, in_=bk_ext[m * 128:(m + 1) * 128].rearrange("(p o) -> p o", o=1)
            )
            bk_col.append(t)
        bv_f32 = singles.tile([1, DPC], F32, tag="bv_f32")
        nc.sync.dma_start(out=bv_f32, in_=bv_ext.rearrange("(o n) -> o n", o=1))
        bv_row = singles.tile([1, DPC], BF16, tag="bv_row")
        nc.vector.tensor_copy(bv_row, bv_f32)


        xT = [singles.tile([128, S], BF16, tag=f"xT{j}", name=f"xT{j}") for j in range(KD)]
        for j in range(KD):
            # split each chunk across both HWDGE rings (SP + ACT) --
            # a single ring's descriptor dispatch caps DMA throughput
            nc.sync.dma_start(out=xT[j][:, 0:S // 2], in_=xt_ext[j * 128:(j + 1) * 128, 0:S // 2])
            nc.scalar.dma_start(out=xT[j][:, S // 2:], in_=xt_ext[j * 128:(j + 1) * 128, S // 2:])

        w_bf = {}
        for name, ext in (("wv", wv_ext), ("wq", wq_ext), ("wk", wk_ext)):
            tiles = []
            for k in range(KD):
                wb = singles.tile([128, DPC], BF16, tag=f"{name}_bf{k}", name=f"{name}_bf{k}")
                # alternate the two hardware DGE rings; the gpsimd SWDGE
                # queue drains too slowly for the v-projection critical path
                eng = nc.sync if k % 2 == 0 else nc.scalar
                eng.dma_start(out=wb, in_=ext[k * 128:(k + 1) * 128, :])
                tiles.append(wb)
            w_bf[name] = tiles

        identity = singles.tile([128, 128], BF16, tag="identity")
        make_identity(nc, identity)
        ones_row = singles.tile([1, 128], BF16, tag="ones_row")
        nc.vector.memset(ones_row, 1.0)

        # --- persistent sbuf tensors ---
        # qTz[h]: q for head h in its pair-row slot, zeros in the other rows;
        # scores then run as K=128 matmuls against the packed kT pair (the
        # foreign head's k rows hit q zeros) -- K=64 same-row-group matmuls
        # serialize on LDWEIGHTS, full-K rotating weights pipeline cleanly
        qTz = [singles.tile([128, S], BF16, tag=f"qTz{h}", name=f"qTz{h}") for h in range(HPC)]
        for h in range(HPC):
            r = 64 * (1 - (h % 2))  # rows NOT owned by this head
            nc.vector.memset(qTz[h][r:r + 64, :], 0.0)
        kT = [singles.tile([128, S], BF16, tag=f"kT{m}", name=f"kTt{m}") for m in range(MB)]
        vsb = [singles.tile([128, HPC, DH + 1], BF16, tag=f"v{i}", name=f"v{i}") for i in range(NS)]
        out_full = [singles.tile([128, DPC], F32, tag=f"of{i}", name=f"of{i}") for i in range(NS)]

        # --- psum pools: scores 2x[128,1024]f32 (4 banks) + shared
        # accumulator/projection/transpose pool 2x2 banks = 8 banks total ---
        s_psum = ctx.enter_context(tc.tile_pool(name="s_psum", bufs=2, space="PSUM"))
        o_psum = ctx.enter_context(tc.tile_pool(name="o_psum", bufs=2, space="PSUM"))

        e_pool = ctx.enter_context(tc.tile_pool(name="e_pool", bufs=6))
        attn_sb = ctx.enter_context(tc.tile_pool(name="attn_sb", bufs=4))
        ot_sb = ctx.enter_context(tc.tile_pool(name="ot_sb", bufs=8))

        def gen_qk_proj(m, n):
            """Generator: q/k projection group for pair m, 2 matmuls per step."""
            sl = slice(n * 512, (n + 1) * 512)
            ps = o_psum.tile([128, 512], F32, tag="po", name=f"ppq{m}_{n}")
            for k in range(KD):
                nc.tensor.matmul(
                    ps,
                    lhsT=w_bf["wq"][k][:, m * 128:(m + 1) * 128],
                    rhs=xT[k][:, sl],
                    start=(k == 0),
                    stop=(k == KD - 1),
                )
                if k % 2 == 1:
                    yield
            nc.vector.tensor_scalar_add(qTz[2 * m][0:64, sl], ps[0:64, :], bq_col[m][0:64])
            nc.vector.tensor_scalar_add(
                qTz[2 * m + 1][64:128, sl], ps[64:128, :], bq_col[m][64:128]
            )
            ps = o_psum.tile([128, 512], F32, tag="po", name=f"ppk{m}_{n}")
            for k in range(KD):
                nc.tensor.matmul(
                    ps,
                    lhsT=w_bf["wk"][k][:, m * 128:(m + 1) * 128],
                    rhs=xT[k][:, sl],
                    start=(k == 0),
                    stop=(k == KD - 1),
                )
                if k % 2 == 1:
                    yield
            nc.vector.tensor_scalar_add(kT[m][:, sl], ps, bk_col[m])

        def emit_qk_proj(m, n):
            for _ in gen_qk_proj(m, n):
                pass

        def emit_v_proj(i):
            ps = o_psum.tile([128, 512], F32, tag="po", name=f"vp{i}")
            for k in range(KD):
                nc.tensor.matmul(
                    ps,
                    lhsT=xT[k][:, i * 128:(i + 1) * 128],
                    rhs=w_bf["wv"][k],
                    start=(k == 0),
                    stop=False,
                )
            nc.tensor.matmul(ps, lhsT=ones_row, rhs=bv_row, start=False, stop=True)
            nc.vector.tensor_copy(
                vsb[i][:, :, 0:DH], ps.rearrange("p (h d) -> p h d", h=HPC)
            )
            nc.vector.memset(vsb[i][:, :, DH:DH + 1], 1.0)

        def emit_out_dma(i):
            eng = nc.sync if i % 2 == 0 else nc.scalar
            eng.dma_start(out=out_ext[i * 128:(i + 1) * 128, :], in_=out_full[i])

        def emit_head_tail_piece(h, o_sb, c2):
            """Transpose + normalize + write one 128-row slab of head h."""
            pt = o_psum.tile([128, 65], BF16, tag="po", name=f"pt{h}_{c2}")
            nc.tensor.transpose(
                pt, o_sb[:, c2 * 128:(c2 + 1) * 128], identity[0:65, 0:65]
            )
            ot = ot_sb.tile([128, 65], BF16, tag="ot", name=f"ot{h}_{c2}")
            nc.vector.tensor_copy(ot, pt)
            rc = ot_sb.tile([128, 1], F32, tag="rc", name=f"rc{h}_{c2}")
            nc.vector.reciprocal(rc, ot[:, DH:DH + 1])
            nc.vector.tensor_scalar_mul(
                out_full[c2][:, h * DH:(h + 1) * DH], ot[:, 0:DH], rc
            )

        # warm the PE clock (HAM) while DMA streams in: each pulse reads the
        # just-arrived xT chunk so PE activity spans the whole load window
        warm = o_psum.tile([128, 512], F32, tag="po", name="warm")
        for j in range(KD):
            for i in range(5):
                nc.tensor.matmul(
                    warm, lhsT=identity, rhs=xT[j][:, 0:512], start=True, stop=True
                )

        emit_qk_proj(0, 0)
        emit_qk_proj(0, 1)
        emit_qk_proj(0, 2)
        emit_qk_proj(0, 3)

        o_sbs = {}
        for h in range(HPC):
            hp = h // 2
            o_sb = attn_sb.tile([65, S], BF16, tag="o_sb", name=f"osb{h}")
            o_sbs[h] = o_sb
            # interleaved filler for this head's 32 chunk iterations:
            # each item is a small closure emitting a couple of PE ops.
            # qk projections for pair p+1 are split across both units of
            # pair p (groups 0-1 in the even head, 2-3 + swaps in the odd)
            # so neither unit is overloaded.
            filler = []
            if h > 0:
                filler += [
                    (emit_head_tail_piece, (h - 1, o_sbs[h - 1], c2)) for c2 in range(NS)
                ]
            filler_t1 = []
            if h == HPC - 1:
                # first sq-half slabs only need this head's t=0 accumulator:
                # overlap them (and their output DMAs) with the t=1 stream.
                # (kept in a separate list consumed only during t=1 -- they
                # must be emitted after the t=0 copyback for deps to form)
                for c2 in range(NS // 2):
                    filler_t1.append((emit_head_tail_piece, (h, o_sb, c2)))
                    filler_t1.append((emit_out_dma, (c2,)))
            if hp + 1 < MB:
                ns_here = (0, 1) if h % 2 == 0 else (2, 3)
                for n in ns_here:
                    g = gen_qk_proj(hp + 1, n)
                    filler += [(g.__next__, ())] * 8 + [
                        (lambda gg=g: list(gg), ())
                    ]
            fi = 0
            n_iters = NT * NS
            for t in range(NT):
                po = o_psum.tile([128, 1024], F32, tag="po", name=f"po{h}_{t}")
                for c0 in range(0, NS, 2):
                    it = t * NS + c0
                    if filler:
                        # drain filler by ~3/4 through the unit so the next
                        # pair's q/k are ready before its first scores
                        want = min(((it + 2) * len(filler)) // (n_iters - 8), len(filler))
                        while fi < want:
                            fn, args = filler[fi]
                            try:
                                fn(*args)
                            except StopIteration:
                                pass
                            fi += 1
                    if h == 0 and t == 0:
                        emit_v_proj(c0)
                        emit_v_proj(c0 + 1)
                    if t == 1 and filler_t1:
                        want1 = min(((c0 + 2) * len(filler_t1)) // NS, len(filler_t1))
                        while len(filler_t1) and want1 > 0:
                            fn, args = filler_t1.pop(0)
                            fn(*args)
                            want1 -= 1
                    pscs = []
                    es = []
                    for c in (c0, c0 + 1):
                        psc = s_psum.tile([128, 1024], F32, tag="psc", name=f"ps{h}_{t}_{c}")
                        pscs.append(psc)
                        for half in range(2):
                            nc.tensor.matmul(
                                psc[:, half * 512:(half + 1) * 512],
                                lhsT=kT[hp][:, c * 128:(c + 1) * 128],
                                rhs=qTz[h][
                                    :,
                                    t * 1024 + half * 512:t * 1024 + (half + 1) * 512,
                                ],
                                start=True, stop=True,
                            )
                    for i, c in enumerate((c0, c0 + 1)):
                        e = e_pool.tile([128, 1024], BF16, tag="e", name=f"e{h}_{t}_{c}")
                        es.append(e)
                        nc.scalar.activation(
                            e, pscs[i], mybir.ActivationFunctionType.Exp, scale=SCALE
                        )
                    # 4 AV matmuls, K=128 rotating weights -> LDW pipelines
                    for i, c in enumerate((c0, c0 + 1)):
                        for half in range(2):
                            nc.tensor.matmul(
                                po[0:65, half * 512:(half + 1) * 512],
                                lhsT=vsb[c][:, h, :],
                                rhs=es[i][:, half * 512:(half + 1) * 512],
                                start=(c == 0), stop=(c == NS - 1),
                            )
                nc.vector.tensor_copy(o_sb[0:65, t * 1024:(t + 1) * 1024], po[0:65, :])
            while fi < len(filler):
                fn, args = filler[fi]
                try:
                    fn(*args)
                except StopIteration:
                    pass
                fi += 1
        # tail of the last head: second sq-half slabs, DMA as soon as ready
        for c2 in range(NS // 2, NS):
            emit_head_tail_piece(HPC - 1, o_sbs[HPC - 1], c2)
            emit_out_dma(c2)

    nc.compile()
    return nc


def _get_program():
    if "nc" not in _CACHE:
        _CACHE["nc"] = _build_program()
    return _CACHE["nc"]


def kernel(x, Wq, bq, Wk, bk, Wv, bv, _trace=False):
    bf = ml_dtypes.bfloat16
    x = np.asarray(x, dtype=np.float32)
    Wq = np.asarray(Wq, dtype=np.float32)
    Wk = np.asarray(Wk, dtype=np.float32)
    Wv = np.asarray(Wv, dtype=np.float32)
    bq = np.ascontiguousarray(np.asarray(bq, dtype=np.float32))
    bk = np.ascontiguousarray(np.asarray(bk, dtype=np.float32))
    bv = np.ascontiguousarray(np.asarray(bv, dtype=np.float32))

    nc = _get_program()

    in_maps = []
    for c in range(N_CORES):
        b, g = c // 2, c % 2
        cols = slice(g * DPC, (g + 1) * DPC)
        in_maps.append(
            {
                "xt": np.ascontiguousarray(x[b].T.astype(bf)),
                "wq": np.ascontiguousarray(Wq[:, cols].astype(bf)),
                "wk": np.ascontiguousarray(Wk[:, cols].astype(bf)),
                "wv": np.ascontiguousarray(Wv[:, cols].astype(bf)),
                "bq": np.ascontiguousarray(bq[cols]),
                "bk": np.ascontiguousarray(bk[cols]),
                "bv": np.ascontiguousarray(bv[cols]),
            }
        )

    res = run_bass_kernel_spmd(nc, in_maps, core_ids=list(range(N_CORES)), trace=_trace)
    _CACHE["last_results"] = res

    out = np.empty((B, S, D), dtype=np.float32)
    for c in range(N_CORES):
        b, g = c // 2, c % 2
        out[b, :, g * DPC:(g + 1) * DPC] = res.results[c]["out"]
    return out


# revision 32
# speedup vs baseline: 1.0006x; 1.0006x over previous
"""Multi-head attention (B=4, S=2048, D=1024, H=16, Dh=64) on 8 TRN2 NeuronCores.

Sharding: core c -> batch b = c//2, head-group g = c%2 (8 heads, output cols
g*512:(g+1)*512).  Host ships x pre-transposed ([D, S]) and weights in bf16;
each core runs attention for its (batch, 8 heads) slice; host concatenates the
per-core [2048, 512] outputs.

Per-core kernel (bf16 compute, f32 accumulation):
  - qTz = per-head zero-padded q (head rows in its pair slot, zeros in the
    other head's rows): scores run as K=128 matmuls (lhsT = packed kT pair,
    the foreign head's k rows hit q zeros).  K=64 same-row-group matmuls
    serialize on LDWEIGHTS; full-K rotating weights pipeline cleanly.
  - v natural = xT.T @ Wv (+bias via K=1 ones-matmul), augmented with a
    ones-column per head so the AV matmul also produces softmax denominators.
  - per head, per sq-tile: scoresT[sk,sq] K=128 single-shot matmuls into
    double-buffered [128,1024] f32 PSUM; exp on ScalarE (1024 wide, the
    1/sqrt(1024) scale folded into the activation; scores are O(1), no
    max-subtraction needed); out_hT/denoms accumulate in PSUM over sk chunks.
  - PE-transpose [65,128] slabs -> natural [128,64|denom] -> reciprocal +
    per-partition scalar multiply -> out rows.
  - v/qk projections and the previous head's transpose/normalize tail are
    interleaved into the attention chunk stream so the TensorE's slack under
    the ScalarE-bound exp is spent on useful work (and HAM stays warm).
"""

import numpy as np
import ml_dtypes
from contextlib import ExitStack

import concourse.bass as bass
import concourse.bacc as bacc
import concourse.mybir as mybir
import concourse.tile as tile
from concourse.bass_utils import run_bass_kernel_spmd
from concourse.masks import make_identity

F32 = mybir.dt.float32
BF16 = mybir.dt.bfloat16

B, S, D = 4, 2048, 1024
H, DH = 16, 64
N_CORES = 8
HPC = 8          # heads per core
DPC = HPC * DH   # output cols per core = 512
SCALE = 1.0 / 32.0  # 1/sqrt(D)

KD = D // 128    # 8 contraction chunks over d_in
NS = S // 128    # 16 sequence chunks
MB = DPC // 128  # 4 partition blocks (head pairs)
NT = S // 1024   # 2 sq tiles

_CACHE = {}


def _build_program():
    nc = bacc.Bacc("TRN2", target_bir_lowering=False, debug=False)

    xt_ext = nc.dram_tensor("xt", [D, S], BF16, kind="ExternalInput").ap()
    wq_ext = nc.dram_tensor("wq", [D, DPC], BF16, kind="ExternalInput").ap()
    wk_ext = nc.dram_tensor("wk", [D, DPC], BF16, kind="ExternalInput").ap()
    wv_ext = nc.dram_tensor("wv", [D, DPC], BF16, kind="ExternalInput").ap()
    bq_ext = nc.dram_tensor("bq", [DPC], F32, kind="ExternalInput").ap()
    bk_ext = nc.dram_tensor("bk", [DPC], F32, kind="ExternalInput").ap()
    bv_ext = nc.dram_tensor("bv", [DPC], F32, kind="ExternalInput").ap()
    out_ext = nc.dram_tensor("out", [S, DPC], F32, kind="ExternalOutput").ap()

    with tile.TileContext(nc, pool_alloc_mode="queue") as tc, ExitStack() as ctx:
        singles = ctx.enter_context(tc.tile_pool(name="singles", bufs=1))

        # --- DMAs: tiny bias vectors first (they gate the first projection
        # copybacks and would otherwise queue behind 7MB of weights), then x,
        # then weights in use order ---
        bq_col = []
        bk_col = []
        for m in range(MB):
            t = singles.tile([128, 1], F32, tag=f"bq{m}", name=f"bq{m}")
            nc.sync.dma_start(
                out=t, in_=bq_ext[m * 128:(m + 1) * 128].rearrange("(p o) -> p o", o=1)
            )
            bq_col.append(t)
            t = singles.tile([128, 1], F32, tag=f"bk{m}", name=f"bk{m}")
            nc.sync.dma_start(
                out=t, in_=bk_ext[m * 128:(m + 1) * 128].rearrange("(p o) -> p o", o=1)
            )
            bk_col.append(t)
        bv_f32 = singles.tile([1, DPC], F32, tag="bv_f32")
        nc.sync.dma_start(out=bv_f32, in_=bv_ext.rearrange("(o n) -> o n", o=1))
        bv_row = singles.tile([1, DPC], BF16, tag="bv_row")
        nc.vector.tensor_copy(bv_row, bv_f32)


        xT = [singles.tile([128, S], BF16, tag=f"xT{j}", name=f"xT{j}") for j in range(KD)]
        for j in range(KD):
            # split each chunk across both HWDGE rings (SP + ACT) --
            # a single ring's descriptor dispatch caps DMA throughput
            nc.sync.dma_start(out=xT[j][:, 0:S // 2], in_=xt_ext[j * 128:(j + 1) * 128, 0:S // 2])
            nc.scalar.dma_start(out=xT[j][:, S // 2:], in_=xt_ext[j * 128:(j + 1) * 128, S // 2:])

        w_bf = {}
        for name, ext in (("wv", wv_ext), ("wq", wq_ext), ("wk", wk_ext)):
            tiles = []
            for k in range(KD):
                wb = singles.tile([128, DPC], BF16, tag=f"{name}_bf{k}", name=f"{name}_bf{k}")
                # alternate the two hardware DGE rings; the gpsimd SWDGE
                # queue drains too slowly for the v-projection critical path
                eng = nc.sync if k % 2 == 0 else nc.scalar
                eng.dma_start(out=wb, in_=ext[k * 128:(k + 1) * 128, :])
                tiles.append(wb)
            w_bf[name] = tiles

        identity = singles.tile([128, 128], BF16, tag="identity")
        make_identity(nc, identity)
        ones_row = singles.tile([1, 128], BF16, tag="ones_row")
        nc.vector.memset(ones_row, 1.0)

        # --- persistent sbuf tensors ---
        # qTz[h]: q for head h in its pair-row slot, zeros in the other rows;
        # scores then run as K=128 matmuls against the packed kT pair (the
        # foreign head's k rows hit q zeros) -- K=64 same-row-group matmuls
        # serialize on LDWEIGHTS, full-K rotating weights pipeline cleanly
        qTz = [singles.tile([128, S], BF16, tag=f"qTz{h}", name=f"qTz{h}") for h in range(HPC)]
        for h in range(HPC):
            r = 64 * (1 - (h % 2))  # rows NOT owned by this head
            nc.vector.memset(qTz[h][r:r + 64, :], 0.0)
        kT = [singles.tile([128, S], BF16, tag=f"kT{m}", name=f"kTt{m}") for m in range(MB)]
        vsb = [singles.tile([128, HPC, DH + 1], BF16, tag=f"v{i}", name=f"v{i}") for i in range(NS)]
        out_full = [singles.tile([128, DPC], F32, tag=f"of{i}", name=f"of{i}") for i in range(NS)]

        # --- psum pools: scores 2x[128,1024]f32 (4 banks) + shared
        # accumulator/projection/transpose pool 2x2 banks = 8 banks total ---
        s_psum = ctx.enter_context(tc.tile_pool(name="s_psum", bufs=2, space="PSUM"))
        o_psum = ctx.enter_context(tc.tile_pool(name="o_psum", bufs=2, space="PSUM"))

        e_pool = ctx.enter_context(tc.tile_pool(name="e_pool", bufs=6))
        attn_sb = ctx.enter_context(tc.tile_pool(name="attn_sb", bufs=4))
        ot_sb = ctx.enter_context(tc.tile_pool(name="ot_sb", bufs=8))

        def gen_qk_proj(m, n):
            """Generator: q/k projection group for pair m, 2 matmuls per step."""
            sl = slice(n * 512, (n + 1) * 512)
            ps = o_psum.tile([128, 512], F32, tag="po", name=f"ppq{m}_{n}")
            for k in range(KD):
                nc.tensor.matmul(
                    ps,
                    lhsT=w_bf["wq"][k][:, m * 128:(m + 1) * 128],
                    rhs=xT[k][:, sl],
                    start=(k == 0),
                    stop=(k == KD - 1),
                )
                if k % 2 == 1:
                    yield
            nc.vector.tensor_scalar_add(qTz[2 * m][0:64, sl], ps[0:64, :], bq_col[m][0:64])
            nc.vector.tensor_scalar_add(
                qTz[2 * m + 1][64:128, sl], ps[64:128, :], bq_col[m][64:128]
            )
            ps = o_psum.tile([128, 512], F32, tag="po", name=f"ppk{m}_{n}")
            for k in range(KD):
                nc.tensor.matmul(
                    ps,
                    lhsT=w_bf["wk"][k][:, m * 128:(m + 1) * 128],
                    rhs=xT[k][:, sl],
                    start=(k == 0),
                    stop=(k == KD - 1),
                )
                if k % 2 == 1:
                    yield
            nc.vector.tensor_scalar_add(kT[m][:, sl], ps, bk_col[m])

        def emit_qk_proj(m, n):
            for _ in gen_qk_proj(m, n):
                pass

        def emit_v_proj(i):
            ps = o_psum.tile([128, 512], F32, tag="po", name=f"vp{i}")
            for k in range(KD):
                nc.tensor.matmul(
                    ps,
                    lhsT=xT[k][:, i * 128:(i + 1) * 128],
                    rhs=w_bf["wv"][k],
                    start=(k == 0),
                    stop=False,
                )
            nc.tensor.matmul(ps, lhsT=ones_row, rhs=bv_row, start=False, stop=True)
            nc.vector.tensor_copy(
                vsb[i][:, :, 0:DH], ps.rearrange("p (h d) -> p h d", h=HPC)
            )
            nc.vector.memset(vsb[i][:, :, DH:DH + 1], 1.0)

        def emit_out_dma(i):
            eng = nc.sync if i % 2 == 0 else nc.scalar
            eng.dma_start(out=out_ext[i * 128:(i + 1) * 128, :], in_=out_full[i])

        def emit_head_tail_piece(h, o_sb, c2):
            """Transpose + normalize + write one 128-row slab of head h."""
            pt = o_psum.tile([128, 65], BF16, tag="po", name=f"pt{h}_{c2}")
            nc.tensor.transpose(
                pt, o_sb[:, c2 * 128:(c2 + 1) * 128], identity[0:65, 0:65]
            )
            ot = ot_sb.tile([128, 65], BF16, tag="ot", name=f"ot{h}_{c2}")
            nc.vector.tensor_copy(ot, pt)
            rc = ot_sb.tile([128, 1], F32, tag="rc", name=f"rc{h}_{c2}")
            nc.vector.reciprocal(rc, ot[:, DH:DH + 1])
            nc.vector.tensor_scalar_mul(
                out_full[c2][:, h * DH:(h + 1) * DH], ot[:, 0:DH], rc
            )

        # warm the PE clock (HAM) while DMA streams in: each pulse reads the
        # just-arrived xT chunk so PE activity spans the whole load window
        warm = o_psum.tile([128, 512], F32, tag="po", name="warm")
        for j in range(KD):
            for i in range(5):
                nc.tensor.matmul(
                    warm, lhsT=identity, rhs=xT[j][:, 0:512], start=True, stop=True
                )

        emit_qk_proj(0, 0)
        emit_qk_proj(0, 1)
        emit_qk_proj(0, 2)
        emit_qk_proj(0, 3)

        o_sbs = {}
        for h in range(HPC):
            hp = h // 2
            o_sb = attn_sb.tile([65, S], BF16, tag="o_sb", name=f"osb{h}")
            o_sbs[h] = o_sb
            # interleaved filler for this head's 32 chunk iterations:
            # each item is a small closure emitting a couple of PE ops.
            # qk projections for pair p+1 are split across both units of
            # pair p (groups 0-1 in the even head, 2-3 + swaps in the odd)
            # so neither unit is overloaded.
            filler = []
            if h > 0:
                filler += [
                    (emit_head_tail_piece, (h - 1, o_sbs[h - 1], c2)) for c2 in range(NS)
                ]
            filler_t1 = []
            if h == HPC - 1:
                # first sq-half slabs only need this head's t=0 accumulator:
                # overlap them (and their output DMAs) with the t=1 stream.
                # (kept in a separate list consumed only during t=1 -- they
                # must be emitted after the t=0 copyback for deps to form)
                for c2 in range(NS // 2):
                    filler_t1.append((emit_head_tail_piece, (h, o_sb, c2)))
                    filler_t1.append((emit_out_dma, (c2,)))
            if hp + 1 < MB:
                ns_here = (0, 1) if h % 2 == 0 else (2, 3)
                for n in ns_here:
                    g = gen_qk_proj(hp + 1, n)
                    filler += [(g.__next__, ())] * 8 + [
                        (lambda gg=g: list(gg), ())
                    ]
            fi = 0
            n_iters = NT * NS
            for t in range(NT):
                po = o_psum.tile([128, 1024], F32, tag="po", name=f"po{h}_{t}")
                for c0 in range(0, NS, 2):
                    it = t * NS + c0
                    if filler:
                        # drain filler by ~3/4 through the unit so the next
                        # pair's q/k are ready before its first scores
                        want = min(((it + 2) * len(filler)) // (n_iters - 8), len(filler))
                        while fi < want:
                            fn, args = filler[fi]
                            try:
                                fn(*args)
                            except StopIteration:
                                pass
                            fi += 1
                    if h == 0 and t == 0:
                        emit_v_proj(c0)
                        emit_v_proj(c0 + 1)
                    if t == 1 and filler_t1:
                        want1 = min(((c0 + 2) * len(filler_t1)) // NS, len(filler_t1))
                        while len(filler_t1) and want1 > 0:
                            fn, args = filler_t1.pop(0)
                            fn(*args)
                            want1 -= 1
                    pscs = []
                    es = []
                    for c in (c0, c0 + 1):
                        psc = s_psum.tile([128, 1024], F32, tag="psc", name=f"ps{h}_{t}_{c}")
                        pscs.append(psc)
                        for half in range(2):
                            nc.tensor.matmul(
                                psc[:, half * 512:(half + 1) * 512],
                                lhsT=kT[hp][:, c * 128:(c + 1) * 128],
                                rhs=qTz[h][
                                    :,
                                    t * 1024 + half * 512:t * 1024 + (half + 1) * 512,
                                ],
                                start=True, stop=True,
                            )
                    for i, c in enumerate((c0, c0 + 1)):
                        e = e_pool.tile([128, 1024], BF16, tag="e", name=f"e{h}_{t}_{c}")
                        es.append(e)
                        nc.scalar.activation(
                            e, pscs[i], mybir.ActivationFunctionType.Exp, scale=SCALE
                        )
                    # 4 AV matmuls, K=128 rotating weights -> LDW pipelines
                    for i, c in enumerate((c0, c0 + 1)):
                        for half in range(2):
                            nc.tensor.matmul(
                                po[0:65, half * 512:(half + 1) * 512],
                                lhsT=vsb[c][:, h, :],
                                rhs=es[i][:, half * 512:(half + 1) * 512],
                                start=(c == 0), stop=(c == NS - 1),
                            )
                nc.vector.tensor_copy(o_sb[0:65, t * 1024:(t + 1) * 1024], po[0:65, :])
            while fi < len(filler):
                fn, args = filler[fi]
                try:
                    fn(*args)
                except StopIteration:
                    pass
                fi += 1
        # tail of the last head: second sq-half slabs, DMA as soon as ready
        for c2 in range(NS // 2, NS):
            emit_head_tail_piece(HPC - 1, o_sbs[HPC - 1], c2)
            emit_out_dma(c2)

    nc.compile()
    return nc


def _get_program():
    if "nc" not in _CACHE:
        _CACHE["nc"] = _build_program()
    return _CACHE["nc"]


def kernel(x, Wq, bq, Wk, bk, Wv, bv, _trace=False):
    bf = ml_dtypes.bfloat16
    x = np.asarray(x, dtype=np.float32)
    Wq = np.asarray(Wq, dtype=np.float32)
    Wk = np.asarray(Wk, dtype=np.float32)
    Wv = np.asarray(Wv, dtype=np.float32)
    bq = np.ascontiguousarray(np.asarray(bq, dtype=np.float32))
    bk = np.ascontiguousarray(np.asarray(bk, dtype=np.float32))
    bv = np.ascontiguousarray(np.asarray(bv, dtype=np.float32))

    nc = _get_program()

    in_maps = []
    for c in range(N_CORES):
        b, g = c // 2, c % 2
        cols = slice(g * DPC, (g + 1) * DPC)
        in_maps.append(
            {
                "xt": np.ascontiguousarray(x[b].T.astype(bf)),
                "wq": np.ascontiguousarray(Wq[:, cols].astype(bf)),
                "wk": np.ascontiguousarray(Wk[:, cols].astype(bf)),
                "wv": np.ascontiguousarray(Wv[:, cols].astype(bf)),
                "bq": np.ascontiguousarray(bq[cols]),
                "bk": np.ascontiguousarray(bk[cols]),
                "bv": np.ascontiguousarray(bv[cols]),
            }
        )

    res = run_bass_kernel_spmd(nc, in_maps, core_ids=list(range(N_CORES)), trace=_trace)
    _CACHE["last_results"] = res

    out = np.empty((B, S, D), dtype=np.float32)
    for c in range(N_CORES):
        b, g = c // 2, c % 2
        out[b, :, g * DPC:(g + 1) * DPC] = res.results[c]["out"]
    return out


# revision 33
# speedup vs baseline: 1.0010x; 1.0003x over previous
"""Multi-head attention (B=4, S=2048, D=1024, H=16, Dh=64) on 8 TRN2 NeuronCores.

Sharding: core c -> batch b = c//2, head-group g = c%2 (8 heads, output cols
g*512:(g+1)*512).  Host ships x pre-transposed ([D, S]) and weights in bf16;
each core runs attention for its (batch, 8 heads) slice; host concatenates the
per-core [2048, 512] outputs.

Per-core kernel (bf16 compute, f32 accumulation):
  - qTz = per-head zero-padded q (head rows in its pair slot, zeros in the
    other head's rows): scores run as K=128 matmuls (lhsT = packed kT pair,
    the foreign head's k rows hit q zeros).  K=64 same-row-group matmuls
    serialize on LDWEIGHTS; full-K rotating weights pipeline cleanly.
  - v natural = xT.T @ Wv (+bias via K=1 ones-matmul), augmented with a
    ones-column per head so the AV matmul also produces softmax denominators.
  - per head, per sq-tile: scoresT[sk,sq] K=128 single-shot matmuls into
    double-buffered [128,1024] f32 PSUM; exp on ScalarE (1024 wide, the
    1/sqrt(1024) scale folded into the activation; scores are O(1), no
    max-subtraction needed); out_hT/denoms accumulate in PSUM over sk chunks.
  - PE-transpose [65,128] slabs -> natural [128,64|denom] -> reciprocal +
    per-partition scalar multiply -> out rows.
  - v/qk projections and the previous head's transpose/normalize tail are
    interleaved into the attention chunk stream so the TensorE's slack under
    the ScalarE-bound exp is spent on useful work (and HAM stays warm).
"""

import numpy as np
import ml_dtypes
from contextlib import ExitStack

import concourse.bass as bass
import concourse.bacc as bacc
import concourse.mybir as mybir
import concourse.tile as tile
from concourse.bass_utils import run_bass_kernel_spmd
from concourse.masks import make_identity

F32 = mybir.dt.float32
BF16 = mybir.dt.bfloat16

B, S, D = 4, 2048, 1024
H, DH = 16, 64
N_CORES = 8
HPC = 8          # heads per core
DPC = HPC * DH   # output cols per core = 512
SCALE = 1.0 / 32.0  # 1/sqrt(D)

KD = D // 128    # 8 contraction chunks over d_in
NS = S // 128    # 16 sequence chunks
MB = DPC // 128  # 4 partition blocks (head pairs)
NT = S // 1024   # 2 sq tiles

_CACHE = {}


def _build_program():
    nc = bacc.Bacc("TRN2", target_bir_lowering=False, debug=False)

    xt_ext = nc.dram_tensor("xt", [D, S], BF16, kind="ExternalInput").ap()
    wq_ext = nc.dram_tensor("wq", [D, DPC], BF16, kind="ExternalInput").ap()
    wk_ext = nc.dram_tensor("wk", [D, DPC], BF16, kind="ExternalInput").ap()
    wv_ext = nc.dram_tensor("wv", [D, DPC], BF16, kind="ExternalInput").ap()
    bq_ext = nc.dram_tensor("bq", [DPC], F32, kind="ExternalInput").ap()
    bk_ext = nc.dram_tensor("bk", [DPC], F32, kind="ExternalInput").ap()
    bv_ext = nc.dram_tensor("bv", [DPC], F32, kind="ExternalInput").ap()
    out_ext = nc.dram_tensor("out", [S, DPC], F32, kind="ExternalOutput").ap()

    with tile.TileContext(nc, pool_alloc_mode="queue") as tc, ExitStack() as ctx:
        singles = ctx.enter_context(tc.tile_pool(name="singles", bufs=1))

        # --- DMAs: tiny bias vectors first (they gate the first projection
        # copybacks and would otherwise queue behind 7MB of weights), then x,
        # then weights in use order ---
        bq_col = []
        bk_col = []
        for m in range(MB):
            t = singles.tile([128, 1], F32, tag=f"bq{m}", name=f"bq{m}")
            nc.sync.dma_start(
                out=t, in_=bq_ext[m * 128:(m + 1) * 128].rearrange("(p o) -> p o", o=1)
            )
            bq_col.append(t)
            t = singles.tile([128, 1], F32, tag=f"bk{m}", name=f"bk{m}")
            nc.sync.dma_start(
                out=t, in_=bk_ext[m * 128:(m + 1) * 128].rearrange("(p o) -> p o", o=1)
            )
            bk_col.append(t)
        bv_f32 = singles.tile([1, DPC], F32, tag="bv_f32")
        nc.sync.dma_start(out=bv_f32, in_=bv_ext.rearrange("(o n) -> o n", o=1))
        bv_row = singles.tile([1, DPC], BF16, tag="bv_row")
        nc.vector.tensor_copy(bv_row, bv_f32)


        xT = [singles.tile([128, S], BF16, tag=f"xT{j}", name=f"xT{j}") for j in range(KD)]
        for j in range(KD):
            # split each chunk across both HWDGE rings (SP + ACT) --
            # a single ring's descriptor dispatch caps DMA throughput
            nc.sync.dma_start(out=xT[j][:, 0:S // 2], in_=xt_ext[j * 128:(j + 1) * 128, 0:S // 2])
            nc.scalar.dma_start(out=xT[j][:, S // 2:], in_=xt_ext[j * 128:(j + 1) * 128, S // 2:])

        w_bf = {}
        for name, ext in (("wv", wv_ext), ("wq", wq_ext), ("wk", wk_ext)):
            tiles = []
            for k in range(KD):
                wb = singles.tile([128, DPC], BF16, tag=f"{name}_bf{k}", name=f"{name}_bf{k}")
                # alternate the two hardware DGE rings; the gpsimd SWDGE
                # queue drains too slowly for the v-projection critical path
                eng = nc.sync if k % 2 == 0 else nc.scalar
                eng.dma_start(out=wb, in_=ext[k * 128:(k + 1) * 128, :])
                tiles.append(wb)
            w_bf[name] = tiles

        identity = singles.tile([128, 128], BF16, tag="identity")
        make_identity(nc, identity)
        ones_row = singles.tile([1, 128], BF16, tag="ones_row")
        nc.vector.memset(ones_row, 1.0)

        # --- persistent sbuf tensors ---
        # qTz[h]: q for head h in its pair-row slot, zeros in the other rows;
        # scores then run as K=128 matmuls against the packed kT pair (the
        # foreign head's k rows hit q zeros) -- K=64 same-row-group matmuls
        # serialize on LDWEIGHTS, full-K rotating weights pipeline cleanly
        qTz = [singles.tile([128, S], BF16, tag=f"qTz{h}", name=f"qTz{h}") for h in range(HPC)]
        for h in range(HPC):
            r = 64 * (1 - (h % 2))  # rows NOT owned by this head
            nc.vector.memset(qTz[h][r:r + 64, :], 0.0)
        kT = [singles.tile([128, S], BF16, tag=f"kT{m}", name=f"kTt{m}") for m in range(MB)]
        vsb = [singles.tile([128, HPC, DH + 1], BF16, tag=f"v{i}", name=f"v{i}") for i in range(NS)]
        out_full = [singles.tile([128, DPC], F32, tag=f"of{i}", name=f"of{i}") for i in range(NS)]

        # --- psum pools: scores 2x[128,1024]f32 (4 banks) + shared
        # accumulator/projection/transpose pool 2x2 banks = 8 banks total ---
        o_psum = ctx.enter_context(tc.tile_pool(name="o_psum", bufs=2, space="PSUM"))
        s_psum = ctx.enter_context(tc.tile_pool(name="s_psum", bufs=2, space="PSUM"))

        e_pool = ctx.enter_context(tc.tile_pool(name="e_pool", bufs=6))
        attn_sb = ctx.enter_context(tc.tile_pool(name="attn_sb", bufs=4))
        ot_sb = ctx.enter_context(tc.tile_pool(name="ot_sb", bufs=8))

        def gen_qk_proj(m, n):
            """Generator: q/k projection group for pair m, 2 matmuls per step."""
            sl = slice(n * 512, (n + 1) * 512)
            ps = o_psum.tile([128, 512], F32, tag="po", name=f"ppq{m}_{n}")
            for k in range(KD):
                nc.tensor.matmul(
                    ps,
                    lhsT=w_bf["wq"][k][:, m * 128:(m + 1) * 128],
                    rhs=xT[k][:, sl],
                    start=(k == 0),
                    stop=(k == KD - 1),
                )
                if k % 2 == 1:
                    yield
            nc.vector.tensor_scalar_add(qTz[2 * m][0:64, sl], ps[0:64, :], bq_col[m][0:64])
            nc.vector.tensor_scalar_add(
                qTz[2 * m + 1][64:128, sl], ps[64:128, :], bq_col[m][64:128]
            )
            ps = o_psum.tile([128, 512], F32, tag="po", name=f"ppk{m}_{n}")
            for k in range(KD):
                nc.tensor.matmul(
                    ps,
                    lhsT=w_bf["wk"][k][:, m * 128:(m + 1) * 128],
                    rhs=xT[k][:, sl],
                    start=(k == 0),
                    stop=(k == KD - 1),
                )
                if k % 2 == 1:
                    yield
            nc.vector.tensor_scalar_add(kT[m][:, sl], ps, bk_col[m])

        def emit_qk_proj(m, n):
            for _ in gen_qk_proj(m, n):
                pass

        def emit_v_proj(i):
            ps = o_psum.tile([128, 512], F32, tag="po", name=f"vp{i}")
            for k in range(KD):
                nc.tensor.matmul(
                    ps,
                    lhsT=xT[k][:, i * 128:(i + 1) * 128],
                    rhs=w_bf["wv"][k],
                    start=(k == 0),
                    stop=False,
                )
            nc.tensor.matmul(ps, lhsT=ones_row, rhs=bv_row, start=False, stop=True)
            nc.vector.tensor_copy(
                vsb[i][:, :, 0:DH], ps.rearrange("p (h d) -> p h d", h=HPC)
            )
            nc.vector.memset(vsb[i][:, :, DH:DH + 1], 1.0)

        def emit_out_dma(i):
            eng = nc.sync if i % 2 == 0 else nc.scalar
            eng.dma_start(out=out_ext[i * 128:(i + 1) * 128, :], in_=out_full[i])

        def emit_head_tail_piece(h, o_sb, c2):
            """Transpose + normalize + write one 128-row slab of head h."""
            pt = o_psum.tile([128, 65], BF16, tag="po", name=f"pt{h}_{c2}")
            nc.tensor.transpose(
                pt, o_sb[:, c2 * 128:(c2 + 1) * 128], identity[0:65, 0:65]
            )
            ot = ot_sb.tile([128, 65], BF16, tag="ot", name=f"ot{h}_{c2}")
            nc.vector.tensor_copy(ot, pt)
            rc = ot_sb.tile([128, 1], F32, tag="rc", name=f"rc{h}_{c2}")
            nc.vector.reciprocal(rc, ot[:, DH:DH + 1])
            nc.vector.tensor_scalar_mul(
                out_full[c2][:, h * DH:(h + 1) * DH], ot[:, 0:DH], rc
            )

        # warm the PE clock (HAM) while DMA streams in: each pulse reads the
        # just-arrived xT chunk so PE activity spans the whole load window
        warm = o_psum.tile([128, 512], F32, tag="po", name="warm")
        for j in range(KD):
            for i in range(5):
                nc.tensor.matmul(
                    warm, lhsT=identity, rhs=xT[j][:, 0:512], start=True, stop=True
                )

        emit_qk_proj(0, 0)
        emit_qk_proj(0, 1)
        emit_qk_proj(0, 2)
        emit_qk_proj(0, 3)

        o_sbs = {}
        for h in range(HPC):
            hp = h // 2
            o_sb = attn_sb.tile([65, S], BF16, tag="o_sb", name=f"osb{h}")
            o_sbs[h] = o_sb
            # interleaved filler for this head's 32 chunk iterations:
            # each item is a small closure emitting a couple of PE ops.
            # qk projections for pair p+1 are split across both units of
            # pair p (groups 0-1 in the even head, 2-3 + swaps in the odd)
            # so neither unit is overloaded.
            filler = []
            if h > 0:
                filler += [
                    (emit_head_tail_piece, (h - 1, o_sbs[h - 1], c2)) for c2 in range(NS)
                ]
            filler_t1 = []
            if h == HPC - 1:
                # first sq-half slabs only need this head's t=0 accumulator:
                # overlap them (and their output DMAs) with the t=1 stream.
                # (kept in a separate list consumed only during t=1 -- they
                # must be emitted after the t=0 copyback for deps to form)
                for c2 in range(NS // 2):
                    filler_t1.append((emit_head_tail_piece, (h, o_sb, c2)))
                    filler_t1.append((emit_out_dma, (c2,)))
            if hp + 1 < MB:
                ns_here = (0, 1) if h % 2 == 0 else (2, 3)
                for n in ns_here:
                    g = gen_qk_proj(hp + 1, n)
                    filler += [(g.__next__, ())] * 8 + [
                        (lambda gg=g: list(gg), ())
                    ]
            fi = 0
            n_iters = NT * NS
            for t in range(NT):
                po = o_psum.tile([128, 1024], F32, tag="po", name=f"po{h}_{t}")
                for c0 in range(0, NS, 2):
                    it = t * NS + c0
                    if filler:
                        # drain filler by ~3/4 through the unit so the next
                        # pair's q/k are ready before its first scores
                        want = min(((it + 2) * len(filler)) // (n_iters - 8), len(filler))
                        while fi < want:
                            fn, args = filler[fi]
                            try:
                                fn(*args)
                            except StopIteration:
                                pass
                            fi += 1
                    if h == 0 and t == 0:
                        emit_v_proj(c0)
                        emit_v_proj(c0 + 1)
                    if t == 1 and filler_t1:
                        want1 = min(((c0 + 2) * len(filler_t1)) // NS, len(filler_t1))
                        while len(filler_t1) and want1 > 0:
                            fn, args = filler_t1.pop(0)
                            fn(*args)
                            want1 -= 1
                    pscs = []
                    es = []
                    for c in (c0, c0 + 1):
                        psc = s_psum.tile([128, 1024], F32, tag="psc", name=f"ps{h}_{t}_{c}")
                        pscs.append(psc)
                        for half in range(2):
                            nc.tensor.matmul(
                                psc[:, half * 512:(half + 1) * 512],
                                lhsT=kT[hp][:, c * 128:(c + 1) * 128],
                                rhs=qTz[h][
                                    :,
                                    t * 1024 + half * 512:t * 1024 + (half + 1) * 512,
                                ],
                                start=True, stop=True,
                            )
                    for i, c in enumerate((c0, c0 + 1)):
                        e = e_pool.tile([128, 1024], BF16, tag="e", name=f"e{h}_{t}_{c}")
                        es.append(e)
                        nc.scalar.activation(
                            e, pscs[i], mybir.ActivationFunctionType.Exp, scale=SCALE
                        )
                    # 4 AV matmuls, K=128 rotating weights -> LDW pipelines
                    for i, c in enumerate((c0, c0 + 1)):
                        for half in range(2):
                            nc.tensor.matmul(
                                po[0:65, half * 512:(half + 1) * 512],
                                lhsT=vsb[c][:, h, :],
                                rhs=es[i][:, half * 512:(half + 1) * 512],
                                start=(c == 0), stop=(c == NS - 1),
                            )
                nc.vector.tensor_copy(o_sb[0:65, t * 1024:(t + 1) * 1024], po[0:65, :])
            while fi < len(filler):
                fn, args = filler[fi]
                try:
                    fn(*args)
                except StopIteration:
                    pass
                fi += 1
        # tail of the last head: second sq-half slabs, DMA as soon as ready
        for c2 in range(NS // 2, NS):
            emit_head_tail_piece(HPC - 1, o_sbs[HPC - 1], c2)
            emit_out_dma(c2)

    nc.compile()
    return nc


def _get_program():
    if "nc" not in _CACHE:
        _CACHE["nc"] = _build_program()
    return _CACHE["nc"]


def kernel(x, Wq, bq, Wk, bk, Wv, bv, _trace=False):
    bf = ml_dtypes.bfloat16
    x = np.asarray(x, dtype=np.float32)
    Wq = np.asarray(Wq, dtype=np.float32)
    Wk = np.asarray(Wk, dtype=np.float32)
    Wv = np.asarray(Wv, dtype=np.float32)
    bq = np.ascontiguousarray(np.asarray(bq, dtype=np.float32))
    bk = np.ascontiguousarray(np.asarray(bk, dtype=np.float32))
    bv = np.ascontiguousarray(np.asarray(bv, dtype=np.float32))

    nc = _get_program()

    in_maps = []
    for c in range(N_CORES):
        b, g = c // 2, c % 2
        cols = slice(g * DPC, (g + 1) * DPC)
        in_maps.append(
            {
                "xt": np.ascontiguousarray(x[b].T.astype(bf)),
                "wq": np.ascontiguousarray(Wq[:, cols].astype(bf)),
                "wk": np.ascontiguousarray(Wk[:, cols].astype(bf)),
                "wv": np.ascontiguousarray(Wv[:, cols].astype(bf)),
                "bq": np.ascontiguousarray(bq[cols]),
                "bk": np.ascontiguousarray(bk[cols]),
                "bv": np.ascontiguousarray(bv[cols]),
            }
        )

    res = run_bass_kernel_spmd(nc, in_maps, core_ids=list(range(N_CORES)), trace=_trace)
    _CACHE["last_results"] = res

    out = np.empty((B, S, D), dtype=np.float32)
    for c in range(N_CORES):
        b, g = c // 2, c % 2
        out[b, :, g * DPC:(g + 1) * DPC] = res.results[c]["out"]
    return out
